# revision 1
# baseline (speedup 1.0000x reference)
"""Paged KV-cache decode attention with ALiBi (Baichuan-style), fused
QKV + attention + output projection, tensor-parallel over heads across
8 Trainium2 NeuronCores.

v6: fp16 everywhere + ALiBi window truncation with balanced head
permutation.

ALiBi slopes decay geometrically, so for most heads only the last
`win_h = ceil(40/slope_h)` positions can contribute: everything earlier
has additive bias < -40, i.e. softmax weight < e^-34 ~ 1e-15 of the
winner. We therefore (a) rank all 40 heads by window size, (b) give
each core one head from each octile (balanced shards: every core gets
the same per-slot chunk capacities, so one shared SPMD program works),
and (c) host-pack ONLY the needed trailing chunks of K^T/V per
(slot, seq). This cuts K/V HBM traffic AND the attention matmul count
by ~45% on every core.

Layout strategy (per core, 5 head-slots):
  - qT/kT computed as [640, 4] (head-dim on partitions) so scores matmuls
    need no transposes and the K-cache new-token scatter is a same-partition
    SBUF copy.
  - v computed as [4, 640] (natural) so the V new-token scatter is a tiny
    SBUF->SBUF DMA row write (scalar/HWDGE queue, grouped per head).
  - K packed host-side as [128(d), sum_chunks*128] (K^T), V as
    [128(t%128), sum_chunks, 128(d)]; ONE DMA per head-slot, single-use
    exact-size SBUF tiles (no pool-reuse waits in the DMA stream).
  - One explicitly-ordered bulk DMA stream on the gpsimd queue:
    weights first (the last v-weight chunk gates all attention), then
    K/V slots, then o_proj weights (consumed at DMA pace at the end).
  - Per-head attention emits all 4 seqs' score matmuls before the 4 AV
    chains so the exp round-trips hide behind other seqs' scores.
  - softmax without max-subtraction; masking via host-precomputed
    additive fp32 bias (-1e30).
  - o_proj in natural orientation (lhsT = tiny attn columns, ow rides
    the 512-wide moving side); host sums the 8 partial outputs.
"""

import math
import os
import sys
from contextlib import ExitStack

import numpy as np

sys.path.insert(0, "/opt/trn_rl_repo")

B = 4
E = 5120
H = 40
D = 128
BS = 16
NB = 512
MB = 128
S = MB * BS  # 2048
NCORES = 8
HPC = H // NCORES   # 5 head-slots per core
EPC = HPC * D       # 640

NEG = -1.0e30
GK = 10             # E-chunks (of 128) per qkv weight DMA group
TCUT = 26.0         # alibi bias cutoff: positions with bias < -TCUT dropped
                    # (dropped softmax weight <= ~e^-14 relative: negligible)


def _alibi_slopes(num_heads):
    cp2 = 2 ** int(math.floor(math.log2(num_heads)))
    base = 2.0 ** (-(2.0 ** (-(math.log2(cp2) - 3))))
    slopes = base ** np.arange(1, cp2 + 1, dtype=np.float64)
    if cp2 != num_heads:
        extra_base = 2.0 ** (-(2.0 ** (-(math.log2(2 * cp2) - 3))))
        n_rem = min(cp2, num_heads - cp2)
        extra = extra_base ** np.arange(1, 1 + 2 * n_rem, 2, dtype=np.float64)
        slopes = np.concatenate([slopes, extra])
    return slopes.astype(np.float32)


def _head_partition(pos, nch):
    """Rank heads by alibi window, assign core c slot s <- rank[s*8+c].
    Returns (order, m) where m[s][b] = kept trailing chunks for slot s."""
    win = np.ceil(TCUT / _alibi_slopes(H).astype(np.float64)).astype(np.int64)
    order = np.argsort(win, kind="stable")
    m = []
    for s in range(HPC):
        wmax = int(win[order[s * NCORES:(s + 1) * NCORES]].max())
        m.append(tuple(nch[b] - max(0, (pos[b] - wmax) // 128) for b in range(B)))
    return order, tuple(m)


_PROGRAM_CACHE = {}
LAST_RESULTS = None  # BassKernelResults of the most recent run (for test.py)


def _build_program(pos, nch, m):
    """Build the SPMD Bass program. pos/nch/m are baked statically (same
    for all cores; per-core data varies only via inputs)."""
    import concourse.bacc as bacc
    import concourse.bass as bass
    import concourse.tile as tile
    from concourse import mybir

    f32 = mybir.dt.float32
    f16 = mybir.dt.float16
    nc = bacc.Bacc()

    scnt = [sum(m[s]) for s in range(HPC)]   # chunks per slot
    soff = [0]
    for s in range(HPC):
        soff.append(soff[-1] + scnt[s])
    KCH = soff[-1]
    # chunk offset of (s, b) within slot s's tile
    moff = [[sum(m[s][:b]) for b in range(B)] for s in range(HPC)]
    c0 = [[nch[b] - m[s][b] for b in range(B)] for s in range(HPC)]

    hT = nc.declare_dram_parameter("hT", [128, 40 * B], f16, isOutput=False)
    qkvw = nc.declare_dram_parameter("qkvw", [3, 128, 40, EPC], f16, isOutput=False)
    ow = nc.declare_dram_parameter("ow", [128, HPC * E], f16, isOutput=False)
    kt = nc.declare_dram_parameter("kt", [128, KCH * 128], f16, isOutput=False)
    vt = nc.declare_dram_parameter("vt", [128, KCH, D], f16, isOutput=False)
    bias = nc.declare_dram_parameter("bias", [128, B * HPC * 16], f32, isOutput=False)
    outT = nc.declare_dram_parameter("outT", [B, E], f32, isOutput=True)

    NG = 40 // GK  # weight DMA groups per tensor

    with tile.TileContext(nc) as tc, ExitStack() as ctx:
        consts = ctx.enter_context(tc.tile_pool(name="consts", bufs=1))
        wpool = ctx.enter_context(tc.tile_pool(name="wpool", bufs=4))
        tmp = ctx.enter_context(tc.tile_pool(name="tmp", bufs=4))
        psum = ctx.enter_context(tc.tile_pool(name="psum", bufs=8, space="PSUM"))

        hT_sb = consts.tile([128, 40 * B], f16)          # (E%128, (Echunk, b))
        bias_sb = consts.tile([128, B * HPC * 16], f32)  # (t%128, (b, s, chunk))
        ow_sb = consts.tile([128, HPC * E], f16)
        qT_sb = consts.tile([128, HPC * B], f16)   # col = s*B + b ; partition = d
        kT_sb = consts.tile([128, HPC * B], f16)
        v_sb = consts.tile([B, EPC], f16)          # natural v rows
        colsum_sb = consts.tile([128, HPC * B], f32)
        aoT_sb = consts.tile([128, HPC * B], f32)  # unnormalized attn@V ^T
        out_sb = consts.tile([B, E], f32)          # natural o_proj output

        ones_col = consts.tile([128, 1], f32)
        nc.vector.memset(ones_col[:], 1.0)
        ones_row = consts.tile([1, 128], f32)
        nc.vector.memset(ones_row[:], 1.0)

        # per-slot single-use exact-size K/V tiles (no pool-reuse waits)
        Kts = [consts.tile([128, scnt[s] * 128], f16, name=f"K{s}") for s in range(HPC)]
        Vts = [consts.tile([128, scnt[s], D], f16, name=f"V{s}") for s in range(HPC)]

        # ---- the bulk DMA stream: ONE queue (gpsimd/SWDGE), explicitly
        # ordered. All weight groups land before the K/V slots (the last
        # v-weight chunk gates the whole attention phase); ow last so the
        # o_proj tail is DMA-paced.
        wq, wk, wv = [], [], []

        def qkv_group(w, lst):
            t = wpool.tile([128, GK, EPC], f16, tag="w", name=f"w{w}_{len(lst)}")
            nc.gpsimd.dma_start(
                out=t[:], in_=qkvw[w, :, len(lst) * GK:(len(lst) + 1) * GK, :]
            )
            lst.append(t)

        nc.gpsimd.dma_start(out=hT_sb[:], in_=hT[:])
        nc.gpsimd.dma_start(out=bias_sb[:], in_=bias[:])
        for g in range(NG):
            qkv_group(0, wq)
        nc.gpsimd.dma_start(out=Kts[0][:], in_=kt[:, soff[0] * 128: soff[1] * 128])
        for g in range(NG):
            qkv_group(1, wk)
        # all K tiles right after the k-weights (only ~3MB now): the DVE
        # K-scatters -- which Tile hoists ahead of the bias-adds -- unblock
        # as soon as kT is ready instead of gating on late K arrivals.
        for s in range(1, HPC):
            nc.gpsimd.dma_start(out=Kts[s][:], in_=kt[:, soff[s] * 128: soff[s + 1] * 128])
        for g in range(NG):
            qkv_group(2, wv)
        for s in range(HPC):
            nc.gpsimd.dma_start(out=Vts[s][:], in_=vt[:, soff[s]: soff[s + 1], :])
        # ow in jg-major chunks: o_proj group jg only needs chunk jg//2,
        # so the projection pipelines at DMA pace behind the stream tail.
        for oc in range(5):
            w = HPC * E // 5
            nc.gpsimd.dma_start(out=ow_sb[:, oc * w:(oc + 1) * w], in_=ow[:, oc * w:(oc + 1) * w])

        # ---- fused QKV projection ----
        # q,k transposed orientation: psum[oc] [128, B] accumulated over 40
        # E-chunks; lhsT = W chunk [128(E), 128(outcol)], rhs = hT chunk [128(E), B].
        for w, lst in ((0, wq), (1, wk)):  # 0=q (pre-scaled on host), 1=k
            dst = qT_sb if w == 0 else kT_sb
            ps = [psum.tile([128, B], f32, tag="ps", name=f"ps_qk{w}_{i}") for i in range(HPC)]
            for g in range(NG):
                wt = lst[g]
                for oc in range(HPC):
                    for kl in range(GK):
                        kc = g * GK + kl
                        nc.tensor.matmul(
                            ps[oc][:],
                            lhsT=wt[:, kl, oc * 128:(oc + 1) * 128],
                            rhs=hT_sb[:, kc * B:(kc + 1) * B],
                            start=(kc == 0),
                            stop=(kc == 39),
                        )
            for oc in range(HPC):
                nc.scalar.copy(dst[:, oc * B:(oc + 1) * B], ps[oc][:])

        # v natural orientation: psum [B, 640] (two banks: 512 + 128),
        # lhsT = hT chunk [128(E), B], rhs = Wv chunk [128(E), 640].
        v_ps0 = psum.tile([B, 512], f32, tag="ps")
        v_ps1 = psum.tile([B, EPC - 512], f32, tag="ps")
        for g in range(NG):
            wt = wv[g]
            for kl in range(GK):
                kc = g * GK + kl
                nc.tensor.matmul(
                    v_ps0[:],
                    lhsT=hT_sb[:, kc * B:(kc + 1) * B],
                    rhs=wt[:, kl, :512],
                    start=(kc == 0),
                    stop=(kc == 39),
                )
                nc.tensor.matmul(
                    v_ps1[:],
                    lhsT=hT_sb[:, kc * B:(kc + 1) * B],
                    rhs=wt[:, kl, 512:],
                    start=(kc == 0),
                    stop=(kc == 39),
                )
        nc.scalar.copy(v_sb[:, :512], v_ps0[:])
        nc.scalar.copy(v_sb[:, 512:], v_ps1[:])

        # ---- attention, head-slot-major; per head: scatters, then all 4
        # seqs' scores, then adds/exps, then the 4 AV chains, so the
        # DVE/ACT round-trips hide behind other seqs' score matmuls.
        for s in range(HPC):
            Kt = Kts[s]
            Vt = Vts[s]
            lpos = [moff[s][b] * 128 + (pos[b] // 128 - c0[s][b]) * 128 + pos[b] % 128
                    for b in range(B)]
            # V new-token scatter rows (cross-partition -> DMA). On the sync
            # queue, which is otherwise idle until the final store: the
            # issue waits (v_sb + Vt arrival) can't block exps or adds.
            for b in range(B):
                p = pos[b]
                nc.sync.dma_start(
                    out=Vt[p % 128: p % 128 + 1, moff[s][b] + p // 128 - c0[s][b], :],
                    in_=v_sb[b:b + 1, s * D:(s + 1) * D],
                )
            # K new-token scatter columns (same partitions: DVE copy)
            for b in range(B):
                nc.vector.tensor_copy(
                    Kt[:, lpos[b]: lpos[b] + 1], kT_sb[:, (s * B + b):(s * B + b) + 1]
                )
            sc_ps = [psum.tile([128, 16], f32, tag="ps", name=f"sc_{s}_{b}") for b in range(B)]
            for b in range(B):
                for c in range(m[s][b]):
                    nc.tensor.matmul(
                        sc_ps[b][:, c:c + 1],
                        lhsT=Kt[:, (moff[s][b] + c) * 128:(moff[s][b] + c + 1) * 128],
                        rhs=qT_sb[:, s * B + b: s * B + b + 1],
                        start=True,
                        stop=True,
                    )
            attn = []
            for b in range(B):
                n = m[s][b]
                col = s * B + b
                s_sb = tmp.tile([128, 16], f32, tag="s", name=f"s_{s}_{b}")
                nc.vector.tensor_add(
                    s_sb[:, :n],
                    sc_ps[b][:, :n],
                    bias_sb[:, (b * HPC + s) * 16:(b * HPC + s) * 16 + n],
                )
                attn_sb = tmp.tile([128, 16], f16, tag="attn", name=f"at_{s}_{b}")
                nc.scalar.activation(
                    attn_sb[:, :n],
                    s_sb[:, :n],
                    func=mybir.ActivationFunctionType.Exp,
                    accum_out=colsum_sb[:, col:col + 1],
                )
                attn.append(attn_sb)
            for b in range(B):
                n = m[s][b]
                col = s * B + b
                ao_ps = psum.tile([128, 1], f32, tag="ps", name=f"ao_{s}_{b}")
                for c in range(n):
                    nc.tensor.matmul(
                        ao_ps[:],
                        lhsT=Vt[:, moff[s][b] + c, :],
                        rhs=attn[b][:, c:c + 1],
                        start=(c == 0),
                        stop=(c == n - 1),
                    )
                nc.scalar.copy(aoT_sb[:, col:col + 1], ao_ps[:])

        # ---- softmax normalization (batched over all 20 (s,b)) ----
        sums_ps = psum.tile([1, HPC * B], f32, tag="ps")
        nc.tensor.matmul(
            sums_ps[:], lhsT=ones_col[:], rhs=colsum_sb[:], start=True, stop=True
        )
        recip_sb = tmp.tile([1, HPC * B], f32, tag="recip")
        nc.vector.reciprocal(recip_sb[:], sums_ps[:])
        rb_ps = psum.tile([128, HPC * B], f32, tag="ps")
        nc.tensor.matmul(
            rb_ps[:], lhsT=ones_row[:], rhs=recip_sb[:], start=True, stop=True
        )
        recip_b = tmp.tile([128, HPC * B], f32, tag="recipb")
        nc.vector.tensor_copy(recip_b[:], rb_ps[:])
        attn_nT = consts.tile([128, HPC * B], f16)
        nc.vector.tensor_mul(attn_nT[:], aoT_sb[:], recip_b[:])

        # ---- output projection (natural): out[b, j] ----
        # lhsT = attn_nT slice [128(hd), B] (4-col weight load, ~free);
        # rhs = ow chunk [128(hd%128), 512] moving at 1 col/cycle.
        for jg in range(E // 512):
            ops = psum.tile([B, 512], f32, tag="ps", name=f"ps_o{jg}")
            for hh in range(HPC):
                j0 = jg * (HPC * 512) + hh * 512
                nc.tensor.matmul(
                    ops[:],
                    lhsT=attn_nT[:, hh * B:(hh + 1) * B],
                    rhs=ow_sb[:, j0: j0 + 512],
                    start=(hh == 0),
                    stop=(hh == HPC - 1),
                )
            # alternate evacuation engines so the copy chain pipelines
            if jg % 2 == 0:
                nc.scalar.copy(out_sb[:, jg * 512:(jg + 1) * 512], ops[:])
            else:
                nc.vector.tensor_copy(out_sb[:, jg * 512:(jg + 1) * 512], ops[:])

        nc.sync.dma_start(out=outT[:, :E // 2], in_=out_sb[:, :E // 2])
        nc.sync.dma_start(out=outT[:, E // 2:], in_=out_sb[:, E // 2:])

    nc.compile()  # Bacc finalize: splits multi-waits (matmul 1-wait limit)
    return nc


def _prepare_core_inputs(core, hidden16, qkv16, o16, k16, v16, bt, sl, pos, nch,
                         order, m):
    """Per-core staged arrays with the window-permuted head assignment."""
    heads = [int(order[s * NCORES + core]) for s in range(HPC)]

    # partition-major: qkvw[w, p, kc, c] = W[w, kc*128 + p, head cols c]
    qkvw = np.ascontiguousarray(
        qkv16.reshape(3, E, H, D)[:, :, heads, :]
        .reshape(3, 40, 128, EPC).transpose(0, 2, 1, 3)
    )

    scnt = [sum(m[s]) for s in range(HPC)]
    KCH = sum(scnt)
    moff = [[sum(m[s][:b]) for b in range(B)] for s in range(HPC)]
    soff = [0]
    for s in range(HPC):
        soff.append(soff[-1] + scnt[s])
    c0 = [[nch[b] - m[s][b] for b in range(B)] for s in range(HPC)]

    kg = k16[:, heads]  # [NB, HPC, BS, D]
    vg = v16[:, heads]
    kt = np.zeros((D, KCH * 128), np.float16)
    vt = np.zeros((128, KCH, D), np.float16)
    for b in range(B):
        sd = nch[b] * 128
        blocks = bt[b][: sd // BS]
        kk = kg[blocks].transpose(1, 0, 2, 3).reshape(HPC, sd, D)
        vv = vg[blocks].transpose(1, 0, 2, 3).reshape(HPC, sd, D)
        for s in range(HPC):
            base = soff[s] + moff[s][b]
            n = m[s][b]
            ksl = kk[s, c0[s][b] * 128: sd]              # [n*128, D]
            kt[:, base * 128: (base + n) * 128] = ksl.T
            vt[:, base: base + n, :] = vv[s, c0[s][b] * 128: sd].reshape(
                n, 128, D).transpose(1, 0, 2)

    slopes = _alibi_slopes(H)[heads]
    t_in = np.arange(128)[:, None]
    biasa = np.full((128, B, HPC, 16), NEG, np.float32)
    for b in range(B):
        for s in range(HPC):
            n = m[s][b]
            tg = ((c0[s][b] + np.arange(n))[None, :] * 128 + t_in).astype(np.float32)
            val = slopes[s] * (tg - np.float32(pos[b]))
            val[tg >= sl[b]] = NEG
            biasa[:, b, s, :n] = val

    hTf = np.ascontiguousarray(
        hidden16.T.reshape(40, 128, B).transpose(1, 0, 2).reshape(128, 40 * B)
    )

    # ow pre-transposed, jg-major: owr[p, jg*HPC*512 + s*512 + j'] =
    # o_proj_weight[heads[s]*128 + p, jg*512 + j']
    owr = np.ascontiguousarray(
        o16.reshape(H, D, E)[heads].reshape(HPC, 128, E // 512, 512)
        .transpose(1, 2, 0, 3).reshape(128, HPC * E)
    )

    return dict(
        hT=hTf,
        qkvw=qkvw,
        ow=owr,
        kt=kt,
        vt=vt,
        bias=np.ascontiguousarray(biasa.reshape(128, B * HPC * 16)),
    )


def kernel(**inputs):
    global LAST_RESULTS
    hidden = np.asarray(inputs["hidden_states"], np.float32)
    qkv_w = np.asarray(inputs["qkv_weight"], np.float32)
    o_w = np.asarray(inputs["o_proj_weight"], np.float32)
    k_cache = np.asarray(inputs["k_cache"], np.float32)
    v_cache = np.asarray(inputs["v_cache"], np.float32)
    bt = np.asarray(inputs["block_tables"]).astype(np.int64)
    sl = np.asarray(inputs["sequence_lengths"]).astype(np.int64)

    pos = tuple(int(x) - 1 for x in sl)
    nch = tuple(int(math.ceil(int(x) / 128)) for x in sl)
    order, m = _head_partition(pos, nch)

    # cast once to fp16 (q pre-scaled by 1/sqrt(D) before the cast)
    hidden16 = hidden.astype(np.float16)
    qkv16 = qkv_w.copy()
    qkv16[0] *= np.float32(D ** -0.5)
    qkv16 = qkv16.astype(np.float16)
    o16 = o_w.astype(np.float16)
    k16 = k_cache.astype(np.float16)
    v16 = v_cache.astype(np.float16)

    in_maps = [
        _prepare_core_inputs(c, hidden16, qkv16, o16, k16, v16, bt, sl, pos, nch,
                             order, m)
        for c in range(NCORES)
    ]

    key = (pos, nch, m)
    if key not in _PROGRAM_CACHE:
        _PROGRAM_CACHE[key] = _build_program(pos, nch, m)
    nc = _PROGRAM_CACHE[key]

    from concourse.bass_utils import run_bass_kernel_spmd

    res = run_bass_kernel_spmd(
        nc,
        in_maps,
        core_ids=list(range(NCORES)),
        trace=bool(os.environ.get("BASS_TRACE")),
    )
    LAST_RESULTS = res

    out = np.zeros((B, E), np.float64)
    for c in range(NCORES):
        out += np.asarray(res.results[c]["outT"]).astype(np.float64)
    return out.astype(np.float32)



# revision 5
# speedup vs baseline: 1.1557x; 1.1557x over previous
"""Paged KV-cache decode attention with ALiBi (Baichuan-style), fused
QKV + attention + output projection, tensor-parallel over heads across
8 Trainium2 NeuronCores.

v8: natural-orientation projections + PE transposes, fp8(e3m4) K/V
projection weights, slot-streamed attention/o_proj with split o_proj
tail.

Key structure (per core, 5 head-slots, emission order = descending
alibi window):
  - q/k/v projections computed NATURALLY: lhsT = hT chunk [128(E), 4],
    rhs = W chunk [128(E), 640] streaming -> [4, 640] PSUM. ~11us PE
    each instead of ~28us weight-stationary (LDWEIGHTS-bound, which
    also kept HAM cold at 1.2GHz in the old version).
  - qT/kT obtained via 10 small PE transposes ([4,128] -> [128,4]).
  - wk/wv quantized host-side to fp8 e3m4 with a power-of-2 scale
    (descale folded into the PSUM->SBUF evacuation copy). Error is
    ~9e-3 vs the 2e-2 gate (the k/v path only affects the newly
    decoded token). wq/ow stay fp16.
  - DMA order: hT, bias, wq, wk, Kt(all slots), wv, [Vt(g), ow(g)] per
    slot. Scores chase Kt arrivals; AV chases Vt; o_proj phase-1
    chases ow.
  - V new-token scatters issued early on the sync queue (gated only on
    v evac + Vt arrival).
  - o_proj split: groups jg=0..5 accumulate slot-by-slot in 6 PSUM
    banks as each slot normalizes (start at first slot, stop at last);
    groups 6..9 replay all 5 slots at the end (attn_nT persists).
    Tail after the last normalization is ~5us instead of ~11us.
  - per-slot softmax normalization (sum/reciprocal/broadcast/mul) so
    o_proj phase-1 can start before later slots finish.
"""

import math
import os
import sys
from contextlib import ExitStack

import numpy as np
import ml_dtypes

sys.path.insert(0, "/opt/trn_rl_repo")

B = 4
E = 5120
H = 40
D = 128
BS = 16
NB = 512
MB = 128
S = MB * BS  # 2048
NCORES = 8
HPC = H // NCORES   # 5 head-slots per core
EPC = HPC * D       # 640

NEG = -1.0e30
GK = 10             # E-chunks (of 128) per qkv weight DMA group
NG = 40 // GK
TCUT = 14.0         # alibi bias cutoff (dropped weight <= ~e^-14 rel)
OP1 = 4             # o_proj groups accumulated slot-by-slot (PSUM-live)

E3 = ml_dtypes.float8_e3m4


def _alibi_slopes(num_heads):
    cp2 = 2 ** int(math.floor(math.log2(num_heads)))
    base = 2.0 ** (-(2.0 ** (-(math.log2(cp2) - 3))))
    slopes = base ** np.arange(1, cp2 + 1, dtype=np.float64)
    if cp2 != num_heads:
        extra_base = 2.0 ** (-(2.0 ** (-(math.log2(2 * cp2) - 3))))
        n_rem = min(cp2, num_heads - cp2)
        extra = extra_base ** np.arange(1, 1 + 2 * n_rem, 2, dtype=np.float64)
        slopes = np.concatenate([slopes, extra])
    return slopes.astype(np.float32)


def _head_partition(pos, nch):
    """Rank heads by alibi window, slot g takes ranks [g*8, (g+1)*8).
    Returns (order, m, emit) where m[g][b] = kept trailing chunks and
    emit = slot emission order (descending window)."""
    win = np.ceil(TCUT / _alibi_slopes(H).astype(np.float64)).astype(np.int64)
    order = np.argsort(win, kind="stable")
    m = []
    wmaxs = []
    for g in range(HPC):
        wmax = int(win[order[g * NCORES:(g + 1) * NCORES]].max())
        wmaxs.append(wmax)
        m.append(tuple(nch[b] - max(0, (pos[b] - wmax) // 128) for b in range(B)))
    emit = tuple(sorted(range(HPC), key=lambda g: -wmaxs[g]))
    return order, tuple(m), emit


_PROGRAM_CACHE = {}
LAST_RESULTS = None  # BassKernelResults of the most recent run (for test.py)


def _build_program(pos, nch, m, emit, rs_k, rs_v):
    """Build the SPMD Bass program. pos/nch/m/emit and the fp8 descale
    factors are baked statically (same for all cores)."""
    import concourse.bacc as bacc
    import concourse.bass as bass
    import concourse.tile as tile
    from concourse import mybir
    from concourse.masks import make_identity

    f32 = mybir.dt.float32
    f16 = mybir.dt.float16
    f8 = mybir.dt.float8e3
    nc = bacc.Bacc()

    scnt = [sum(m[g]) for g in range(HPC)]      # chunks per slot
    # Kt/Vt DRAM packing follows emission order
    soff = {}
    acc = 0
    for g in emit:
        soff[g] = acc
        acc += scnt[g]
    KCH = acc
    moff = [[sum(m[g][:b]) for b in range(B)] for g in range(HPC)]
    c0 = [[nch[b] - m[g][b] for b in range(B)] for g in range(HPC)]

    hT = nc.declare_dram_parameter("hT", [128, 40 * B], f16, isOutput=False)
    wq_d = nc.declare_dram_parameter("wq", [128, 40, EPC], f16, isOutput=False)
    wk_d = nc.declare_dram_parameter("wk", [128, 40, EPC], f8, isOutput=False)
    wv_d = nc.declare_dram_parameter("wv", [128, 40, EPC], f8, isOutput=False)
    ow_d = nc.declare_dram_parameter("ow", [HPC, 128, E], f16, isOutput=False)
    kt_d = nc.declare_dram_parameter("kt", [128, KCH * 128], f16, isOutput=False)
    vt_d = nc.declare_dram_parameter("vt", [128, KCH, D], f16, isOutput=False)
    bias_d = nc.declare_dram_parameter("bias", [128, B * HPC * 16], f32, isOutput=False)
    outT = nc.declare_dram_parameter("outT", [B, E], f32, isOutput=True)

    with tile.TileContext(nc) as tc, ExitStack() as ctx:
        consts = ctx.enter_context(tc.tile_pool(name="consts", bufs=1))
        wpool = ctx.enter_context(tc.tile_pool(name="wpool", bufs=4))
        psA = ctx.enter_context(tc.tile_pool(name="psA", bufs=4, space="PSUM"))
        psO = ctx.enter_context(tc.tile_pool(name="psO", bufs=4, space="PSUM"))

        hT_sb = consts.tile([128, 40 * B], f16)
        bias_sb = consts.tile([128, B * HPC * 16], f32)
        ident = consts.tile([B, B], f16)
        make_identity(nc, ident[:])
        ones_col = consts.tile([128, 1], f32)
        nc.vector.memset(ones_col[:], 1.0)
        ones_row = consts.tile([1, 128], f32)
        nc.vector.memset(ones_row[:], 1.0)

        qT_sb = consts.tile([128, HPC * B], f16)    # [d, g*B+b]
        colsum_sb = consts.tile([128, HPC * B], f32)
        aoT_sb = consts.tile([128, HPC * B], f32)
        attn_nT = consts.tile([128, HPC * B], f16)
        q_nat = consts.tile([B, EPC], f16)
        k_nat = consts.tile([B, EPC], f16)
        v_nat = consts.tile([B, EPC], f16)
        out_sb = consts.tile([B, E], f32)

        Kts = [consts.tile([128, scnt[g] * 128], f16, name=f"K{g}") for g in range(HPC)]
        Vts = [consts.tile([128, scnt[g], D], f16, name=f"V{g}") for g in range(HPC)]
        ow_ts = [consts.tile([128, E], f16, name=f"ow{g}") for g in range(HPC)]

        # ---- bulk DMA stream on the gpsimd (SWDGE) queue, in order ----
        nc.gpsimd.dma_start(out=hT_sb[:], in_=hT[:])
        nc.gpsimd.dma_start(out=bias_sb[:], in_=bias_d[:])
        wq_t, wk_t, wv_t = [], [], []

        def wgroup(dram, lst, nm, dt):
            t = wpool.tile([128, GK, EPC], dt, tag="w", name=f"{nm}{len(lst)}")
            nc.gpsimd.dma_start(out=t[:], in_=dram[:, len(lst) * GK:(len(lst) + 1) * GK, :])
            lst.append(t)

        for g in range(NG):
            wgroup(wq_d, wq_t, "wq", f16)
        for g in range(NG):
            wgroup(wk_d, wk_t, "wk", f8)
        for g in emit:
            nc.gpsimd.dma_start(
                out=Kts[g][:], in_=kt_d[:, soff[g] * 128:(soff[g] + scnt[g]) * 128]
            )
        for g in range(NG):
            wgroup(wv_d, wv_t, "wv", f8)
        for g in emit:
            nc.gpsimd.dma_start(
                out=Vts[g][:], in_=vt_d[:, soff[g]:soff[g] + scnt[g], :]
            )
            nc.gpsimd.dma_start(out=ow_ts[g][:], in_=ow_d[emit.index(g), :, :])

        # ---- q/k projections (natural) + transposes ----
        def proj(lst, nat, scale):
            p0 = psA.tile([B, 512], f32, tag="ps")
            p1 = psA.tile([B, EPC - 512], f32, tag="ps")
            for g in range(NG):
                wt = lst[g]
                for kl in range(GK):
                    kc = g * GK + kl
                    nc.tensor.matmul(
                        p0[:], lhsT=hT_sb[:, kc * B:(kc + 1) * B], rhs=wt[:, kl, :512],
                        start=(kc == 0), stop=(kc == 39),
                    )
                    nc.tensor.matmul(
                        p1[:], lhsT=hT_sb[:, kc * B:(kc + 1) * B], rhs=wt[:, kl, 512:],
                        start=(kc == 0), stop=(kc == 39),
                    )
            if scale == 1.0:
                nc.scalar.copy(nat[:, :512], p0[:])
                nc.scalar.copy(nat[:, 512:], p1[:])
            else:
                nc.scalar.mul(nat[:, :512], p0[:], scale)
                nc.scalar.mul(nat[:, 512:], p1[:], scale)

        def transpose_to(nat, dst_sb, g):
            tp = psA.tile([128, B], f16, tag="ps", name=f"tp{g}")
            nc.tensor.transpose(tp[:], nat[:, g * 128:(g + 1) * 128], ident[:])
            if dst_sb is not None:
                nc.scalar.copy(dst_sb[:, g * B:(g + 1) * B], tp[:])
            return tp

        proj(wq_t, q_nat, 1.0)  # q pre-scaled by 1/sqrt(D) host-side
        for g in emit:
            transpose_to(q_nat, qT_sb, g)
        proj(wk_t, k_nat, rs_k)
        for g in emit:
            # kT transpose, then immediately the K new-token scatter
            # (same partitions: DVE copy from PSUM) so the PSUM tile's
            # lifetime stays short.
            kT_ps = transpose_to(k_nat, None, g)
            lp = [moff[g][b] * 128 + (pos[b] // 128 - c0[g][b]) * 128 + pos[b] % 128
                  for b in range(B)]
            for b in range(B):
                nc.vector.tensor_copy(
                    Kts[g][:, lp[b]:lp[b] + 1], kT_ps[:, b:b + 1]
                )

        # ---- per-slot scores + exp (chase Kt arrivals) ----
        attn = {}
        for g in emit:
            Kt = Kts[g]
            sc_ps = [psA.tile([128, 16], f32, tag="ps", name=f"sc_{g}_{b}")
                     for b in range(B)]
            for b in range(B):
                for c in range(m[g][b]):
                    nc.tensor.matmul(
                        sc_ps[b][:, c:c + 1],
                        lhsT=Kt[:, (moff[g][b] + c) * 128:(moff[g][b] + c + 1) * 128],
                        rhs=qT_sb[:, g * B + b:g * B + b + 1],
                        start=True, stop=True,
                    )
            for b in range(B):
                n = m[g][b]
                col = g * B + b
                s_sb = consts.tile([128, 16], f32, name=f"s_{g}_{b}")
                nc.vector.tensor_add(
                    s_sb[:, :n], sc_ps[b][:, :n],
                    bias_sb[:, (b * HPC + g) * 16:(b * HPC + g) * 16 + n],
                )
                a_sb = consts.tile([128, 16], f16, name=f"at_{g}_{b}")
                nc.scalar.activation(
                    a_sb[:, :n], s_sb[:, :n],
                    func=mybir.ActivationFunctionType.Exp,
                    accum_out=colsum_sb[:, col:col + 1],
                )
                attn[(g, b)] = a_sb

        # ---- v projection + per-slot scatter/AV/norm/o_proj-phase-1 ----
        proj(wv_t, v_nat, rs_v)
        opsO = [psO.tile([B, 512], f32, tag="po", name=f"po{jg}") for jg in range(OP1)]
        for gi, g in enumerate(emit):
            Vt = Vts[g]
            for b in range(B):
                p = pos[b]
                nc.sync.dma_start(
                    out=Vt[p % 128:p % 128 + 1, moff[g][b] + p // 128 - c0[g][b], :],
                    in_=v_nat[b:b + 1, g * D:(g + 1) * D],
                )
            for b in range(B):
                n = m[g][b]
                col = g * B + b
                ao_ps = psA.tile([128, 1], f32, tag="ps", name=f"ao_{g}_{b}")
                for c in range(n):
                    nc.tensor.matmul(
                        ao_ps[:],
                        lhsT=Vt[:, moff[g][b] + c, :],
                        rhs=attn[(g, b)][:, c:c + 1],
                        start=(c == 0), stop=(c == n - 1),
                    )
                nc.scalar.copy(aoT_sb[:, col:col + 1], ao_ps[:])
            # per-slot normalization
            sums_ps = psA.tile([1, B], f32, tag="ps", name=f"sum{g}")
            nc.tensor.matmul(
                sums_ps[:], lhsT=ones_col[:],
                rhs=colsum_sb[:, g * B:(g + 1) * B], start=True, stop=True,
            )
            recip_sb = consts.tile([1, B], f32, name=f"rc{g}")
            nc.vector.reciprocal(recip_sb[:], sums_ps[:])
            rb_ps = psA.tile([128, B], f32, tag="ps", name=f"rb{g}")
            nc.tensor.matmul(
                rb_ps[:], lhsT=ones_row[:], rhs=recip_sb[:], start=True, stop=True,
            )
            rb_sb = consts.tile([128, B], f32, name=f"rbs{g}")
            nc.vector.tensor_copy(rb_sb[:], rb_ps[:])
            nc.vector.tensor_mul(
                attn_nT[:, g * B:(g + 1) * B], aoT_sb[:, g * B:(g + 1) * B], rb_sb[:]
            )
            # o_proj phase 1: slot-streamed accumulation
            for jg in range(OP1):
                nc.tensor.matmul(
                    opsO[jg][:],
                    lhsT=attn_nT[:, g * B:(g + 1) * B],
                    rhs=ow_ts[g][:, jg * 512:(jg + 1) * 512],
                    start=(gi == 0), stop=(gi == HPC - 1),
                )

        # ---- o_proj phase 1 evac + phase 2 ----
        for jg in range(OP1):
            if jg % 2 == 0:
                nc.scalar.copy(out_sb[:, jg * 512:(jg + 1) * 512], opsO[jg][:])
            else:
                nc.vector.tensor_copy(out_sb[:, jg * 512:(jg + 1) * 512], opsO[jg][:])
        for jg in range(OP1, E // 512):
            op2 = psO.tile([B, 512], f32, tag="po", name=f"po{jg}")
            for gi, g in enumerate(emit):
                nc.tensor.matmul(
                    op2[:],
                    lhsT=attn_nT[:, g * B:(g + 1) * B],
                    rhs=ow_ts[g][:, jg * 512:(jg + 1) * 512],
                    start=(gi == 0), stop=(gi == HPC - 1),
                )
            if jg % 2 == 0:
                nc.scalar.copy(out_sb[:, jg * 512:(jg + 1) * 512], op2[:])
            else:
                nc.vector.tensor_copy(out_sb[:, jg * 512:(jg + 1) * 512], op2[:])

        nc.sync.dma_start(out=outT[:, :E // 2], in_=out_sb[:, :E // 2])
        nc.sync.dma_start(out=outT[:, E // 2:], in_=out_sb[:, E // 2:])

    nc.compile()
    return nc


def _pow2_scale(x, cap):
    mx = float(np.abs(x).max())
    return 2.0 ** math.floor(math.log2(cap / mx))


def _prepare_core_inputs(core, hidden16, wq16, wk8, wv8, o16, k16, v16, bt, sl,
                         pos, nch, order, m, emit):
    """Per-core staged arrays with the window-permuted head assignment."""
    heads = [int(order[g * NCORES + core]) for g in range(HPC)]

    def wlayout(Wh):  # Wh: [E, H, D] picked heads -> [128, 40, EPC]
        return np.ascontiguousarray(
            Wh.reshape(40, 128, EPC).transpose(1, 0, 2)
        )

    wq = wlayout(wq16.reshape(E, H, D)[:, heads, :].reshape(E, EPC))
    wk = wlayout(wk8.reshape(E, H, D)[:, heads, :].reshape(E, EPC))
    wv = wlayout(wv8.reshape(E, H, D)[:, heads, :].reshape(E, EPC))

    scnt = [sum(m[g]) for g in range(HPC)]
    soff = {}
    acc = 0
    for g in emit:
        soff[g] = acc
        acc += scnt[g]
    KCH = acc
    moff = [[sum(m[g][:b]) for b in range(B)] for g in range(HPC)]
    c0 = [[nch[b] - m[g][b] for b in range(B)] for g in range(HPC)]

    kg = k16[:, heads]  # [NB, HPC, BS, D]
    vg = v16[:, heads]
    kt = np.zeros((D, KCH * 128), np.float16)
    vt = np.zeros((128, KCH, D), np.float16)
    for b in range(B):
        sd = nch[b] * 128
        blocks = bt[b][: sd // BS]
        kk = kg[blocks].transpose(1, 0, 2, 3).reshape(HPC, sd, D)
        vv = vg[blocks].transpose(1, 0, 2, 3).reshape(HPC, sd, D)
        for g in range(HPC):
            base = soff[g] + moff[g][b]
            n = m[g][b]
            ksl = kk[g, c0[g][b] * 128: sd]              # [n*128, D]
            kt[:, base * 128:(base + n) * 128] = ksl.T
            vt[:, base:base + n, :] = vv[g, c0[g][b] * 128: sd].reshape(
                n, 128, D).transpose(1, 0, 2)

    slopes = _alibi_slopes(H)[heads]
    t_in = np.arange(128)[:, None]
    biasa = np.full((128, B, HPC, 16), NEG, np.float32)
    for b in range(B):
        for g in range(HPC):
            n = m[g][b]
            tg = ((c0[g][b] + np.arange(n))[None, :] * 128 + t_in).astype(np.float32)
            val = slopes[g] * (tg - np.float32(pos[b]))
            val[tg >= sl[b]] = NEG
            biasa[:, b, g, :n] = val

    hTf = np.ascontiguousarray(
        hidden16.T.reshape(40, 128, B).transpose(1, 0, 2).reshape(128, 40 * B)
    )

    # ow: [emit-slot-index, 128, E]
    owr = np.ascontiguousarray(o16.reshape(H, D, E)[[heads[g] for g in emit]])

    return dict(
        hT=hTf, wq=wq, wk=wk, wv=wv, ow=owr, kt=kt, vt=vt,
        bias=np.ascontiguousarray(biasa.reshape(128, B * HPC * 16)),
    )


def kernel(**inputs):
    global LAST_RESULTS
    hidden = np.asarray(inputs["hidden_states"], np.float32)
    qkv_w = np.asarray(inputs["qkv_weight"], np.float32)
    o_w = np.asarray(inputs["o_proj_weight"], np.float32)
    k_cache = np.asarray(inputs["k_cache"], np.float32)
    v_cache = np.asarray(inputs["v_cache"], np.float32)
    bt = np.asarray(inputs["block_tables"]).astype(np.int64)
    sl = np.asarray(inputs["sequence_lengths"]).astype(np.int64)

    pos = tuple(int(x) - 1 for x in sl)
    nch = tuple(int(math.ceil(int(x) / 128)) for x in sl)
    order, m, emit = _head_partition(pos, nch)

    hidden16 = hidden.astype(np.float16)
    wq16 = (qkv_w[0] * np.float32(D ** -0.5)).astype(np.float16)
    s_k = _pow2_scale(qkv_w[1], 14.0)
    s_v = _pow2_scale(qkv_w[2], 14.0)
    wk8 = (qkv_w[1] * np.float32(s_k)).astype(E3)
    wv8 = (qkv_w[2] * np.float32(s_v)).astype(E3)
    o16 = o_w.astype(np.float16)
    k16 = k_cache.astype(np.float16)
    v16 = v_cache.astype(np.float16)

    in_maps = [
        _prepare_core_inputs(c, hidden16, wq16, wk8, wv8, o16, k16, v16, bt, sl,
                             pos, nch, order, m, emit)
        for c in range(NCORES)
    ]

    key = (pos, nch, m, emit, s_k, s_v)
    if key not in _PROGRAM_CACHE:
        _PROGRAM_CACHE[key] = _build_program(pos, nch, m, emit, 1.0 / s_k, 1.0 / s_v)
    nc = _PROGRAM_CACHE[key]

    from concourse.bass_utils import run_bass_kernel_spmd

    res = run_bass_kernel_spmd(
        nc,
        in_maps,
        core_ids=list(range(NCORES)),
        trace=bool(os.environ.get("BASS_TRACE")),
    )
    LAST_RESULTS = res

    out = np.zeros((B, E), np.float64)
    for c in range(NCORES):
        out += np.asarray(res.results[c]["outT"]).astype(np.float64)
    return out.astype(np.float32)


# revision 6
# speedup vs baseline: 1.2568x; 1.0874x over previous
"""Paged KV-cache decode attention with ALiBi (Baichuan-style), fused
QKV + attention + output projection, tensor-parallel over heads across
8 Trainium2 NeuronCores.

v9: dense PE schedule + contiguous V new-token scatter.

Per core (5 head-slots, slot index = emission order = descending alibi
window so the small slots land in the tail):
  - all projections in natural orientation (lhsT = hT chunk [128(E),4],
    rhs = W chunk [128(E), 512/128] streaming) -> PE at the stream
    floor (~10.9us each) instead of LDWEIGHTS-bound.
  - emission k -> q -> v keeps the PE on wide streams back-to-back
    (HAM stays at 2.4GHz); scores/AV (N=1 matmuls) chase the K/V tile
    DMAs afterwards, interleaved with N=512 o_proj matmuls.
  - qT/kT via PE transposes ([4,128] -> [128,4]) with prompt PSUM
    evacuation (short tile lifetimes - avoids cross-engine stalls).
  - wk/wv quantized host-side to fp8 e3m4 (power-of-2 scale, descale
    folded into the PSUM->SBUF evac). rel_err ~9e-3 vs 2e-2 gate.
  - V new-token scatter: the last (newest) chunk of every (slot, seq)
    lives in a separate newV tile [128, B*HPC, 128] packed so that the
    new-token row for seq b is CONTIGUOUS -> 4 DMAs total instead of
    20 serial ones (v8 lost ~17us here).
  - o_proj split: 5 groups accumulate slot-by-slot in 5 PSUM banks as
    each slot normalizes (per-slot softmax normalization); 5 groups
    replay all slots at the end. Tail ~6us instead of ~11us.
  - DMA order: hT, bias, wk, wq, wv, Kt(slots), newV, [Vt(i), ow(i)].
"""

import math
import os
import sys
from contextlib import ExitStack

import numpy as np
import ml_dtypes

sys.path.insert(0, "/opt/trn_rl_repo")

B = 4
E = 5120
H = 40
D = 128
BS = 16
NB = 512
MB = 128
S = MB * BS  # 2048
NCORES = 8
HPC = H // NCORES   # 5 head-slots per core
EPC = HPC * D       # 640

NEG = -1.0e30
GK = 10             # E-chunks (of 128) per qkv weight DMA group
NG = 40 // GK
TCUT = 12.0         # alibi bias cutoff (dropped weight <= ~e^-12 rel)
OP1 = 5             # o_proj groups accumulated slot-by-slot (PSUM-live)

E3 = ml_dtypes.float8_e3m4


def _alibi_slopes(num_heads):
    cp2 = 2 ** int(math.floor(math.log2(num_heads)))
    base = 2.0 ** (-(2.0 ** (-(math.log2(cp2) - 3))))
    slopes = base ** np.arange(1, cp2 + 1, dtype=np.float64)
    if cp2 != num_heads:
        extra_base = 2.0 ** (-(2.0 ** (-(math.log2(2 * cp2) - 3))))
        n_rem = min(cp2, num_heads - cp2)
        extra = extra_base ** np.arange(1, 1 + 2 * n_rem, 2, dtype=np.float64)
        slopes = np.concatenate([slopes, extra])
    return slopes.astype(np.float32)


def _head_partition(pos, nch):
    """Rank heads by alibi window; slot i (in emission order, biggest
    window first) gets ranks [gi*8, (gi+1)*8) where gi runs over groups
    sorted by descending window. Returns (head_ranks, m) with
    head_ranks[i] = the 8 global head ids of slot i (per-core pick is
    head_ranks[i][core]) and m[i][b] = kept trailing chunks."""
    win = np.ceil(TCUT / _alibi_slopes(H).astype(np.float64)).astype(np.int64)
    order = np.argsort(win, kind="stable")
    groups = []
    for g in range(HPC):
        ids = order[g * NCORES:(g + 1) * NCORES]
        wmax = int(win[ids].max())
        groups.append((wmax, [int(x) for x in ids]))
    groups.sort(key=lambda t: -t[0])
    head_ranks = tuple(tuple(ids) for _, ids in groups)
    m = tuple(
        tuple(nch[b] - max(0, (pos[b] - wmax) // 128) for b in range(B))
        for wmax, _ in groups
    )
    return head_ranks, m


_PROGRAM_CACHE = {}
LAST_RESULTS = None  # BassKernelResults of the most recent run (for test.py)


def _build_program(pos, nch, m, rs_k, rs_v):
    """Build the SPMD Bass program. pos/nch/m and the fp8 descale
    factors are baked statically (same for all cores)."""
    import concourse.bacc as bacc
    import concourse.bass as bass
    import concourse.tile as tile
    from concourse import mybir
    from concourse.masks import make_identity

    f32 = mybir.dt.float32
    f16 = mybir.dt.float16
    f8 = mybir.dt.float8e3
    nc = bacc.Bacc()

    # Vt holds chunks 0..m-2 per (i,b); the last chunk lives in newV.
    mv = [[m[i][b] - 1 for b in range(B)] for i in range(HPC)]
    vcnt = [sum(mv[i]) for i in range(HPC)]
    kcnt = [sum(m[i]) for i in range(HPC)]
    ksoff = [0]
    vsoff = [0]
    for i in range(HPC):
        ksoff.append(ksoff[-1] + kcnt[i])
        vsoff.append(vsoff[-1] + vcnt[i])
    KCH, VCH = ksoff[-1], vsoff[-1]
    kmoff = [[sum(m[i][:b]) for b in range(B)] for i in range(HPC)]
    vmoff = [[sum(mv[i][:b]) for b in range(B)] for i in range(HPC)]
    c0 = [[nch[b] - m[i][b] for b in range(B)] for i in range(HPC)]

    hT = nc.declare_dram_parameter("hT", [128, 40 * B], f16, isOutput=False)
    wq_d = nc.declare_dram_parameter("wq", [128, 40, EPC], f16, isOutput=False)
    wk_d = nc.declare_dram_parameter("wk", [128, 40, EPC], f8, isOutput=False)
    wv_d = nc.declare_dram_parameter("wv", [128, 40, EPC], f8, isOutput=False)
    ow_d = nc.declare_dram_parameter("ow", [HPC, 128, E], f16, isOutput=False)
    kt_d = nc.declare_dram_parameter("kt", [128, KCH * 128], f16, isOutput=False)
    vt_d = nc.declare_dram_parameter("vt", [128, max(VCH, 1), D], f16, isOutput=False)
    nv_d = nc.declare_dram_parameter("nv", [128, B * HPC, D], f16, isOutput=False)
    bias_d = nc.declare_dram_parameter("bias", [128, B * HPC * 16], f32, isOutput=False)
    outT = nc.declare_dram_parameter("outT", [B, E], f32, isOutput=True)

    with tile.TileContext(nc) as tc, ExitStack() as ctx:
        consts = ctx.enter_context(tc.tile_pool(name="consts", bufs=1))
        wpool = ctx.enter_context(tc.tile_pool(name="wpool", bufs=5))
        psA = ctx.enter_context(tc.tile_pool(name="psA", bufs=3, space="PSUM"))
        psO = ctx.enter_context(tc.tile_pool(name="psO", bufs=5, space="PSUM"))

        hT_sb = consts.tile([128, 40 * B], f16)
        bias_sb = consts.tile([128, B * HPC * 16], f32)
        ident = consts.tile([B, B], f16)
        make_identity(nc, ident[:])
        ones_col = consts.tile([128, 1], f32)
        nc.vector.memset(ones_col[:], 1.0)
        ones_row = consts.tile([1, 128], f32)
        nc.vector.memset(ones_row[:], 1.0)

        qT_sb = consts.tile([128, HPC * B], f16)    # [d, i*B+b]
        kT_sb = consts.tile([128, HPC * B], f16)
        colsum_sb = consts.tile([128, HPC * B], f32)
        aoT_sb = consts.tile([128, HPC * B], f32)
        attn_nT = consts.tile([128, HPC * B], f16)
        q_nat = consts.tile([B, EPC], f16)
        k_nat = consts.tile([B, EPC], f16)
        v_nat = consts.tile([B, EPC], f16)
        out_sb = consts.tile([B, E], f32)

        Kts = [consts.tile([128, kcnt[i] * 128], f16, name=f"K{i}") for i in range(HPC)]
        Vts = [consts.tile([128, max(vcnt[i], 1), D], f16, name=f"V{i}")
               for i in range(HPC)]
        newV = consts.tile([128, B * HPC, D], f16)
        ow_ts = [consts.tile([128, E], f16, name=f"ow{i}") for i in range(HPC)]

        # ---- bulk DMA stream on the gpsimd (SWDGE) queue, in order ----
        nc.gpsimd.dma_start(out=hT_sb[:], in_=hT[:])
        nc.gpsimd.dma_start(out=bias_sb[:], in_=bias_d[:])
        wq_t, wk_t, wv_t = [], [], []

        def wgroup(dram, lst, nm, dt):
            t = wpool.tile([128, GK, EPC], dt, tag="w", name=f"{nm}{len(lst)}")
            nc.gpsimd.dma_start(out=t[:], in_=dram[:, len(lst) * GK:(len(lst) + 1) * GK, :])
            lst.append(t)

        for g in range(NG):
            wgroup(wk_d, wk_t, "wk", f8)
        for g in range(NG):
            wgroup(wq_d, wq_t, "wq", f16)
        for g in range(NG):
            wgroup(wv_d, wv_t, "wv", f8)
        for i in range(HPC):
            nc.gpsimd.dma_start(
                out=Kts[i][:], in_=kt_d[:, ksoff[i] * 128:ksoff[i + 1] * 128]
            )
        nc.gpsimd.dma_start(out=newV[:], in_=nv_d[:])
        for i in range(HPC):
            if vcnt[i]:
                nc.gpsimd.dma_start(
                    out=Vts[i][:], in_=vt_d[:, vsoff[i]:vsoff[i + 1], :]
                )
            nc.gpsimd.dma_start(out=ow_ts[i][:], in_=ow_d[i, :, :])

        # ---- projections (natural orientation) ----
        def proj(lst, nat, scale):
            p0 = psA.tile([B, 512], f32, tag="ps")
            p1 = psA.tile([B, EPC - 512], f32, tag="ps")
            for g in range(NG):
                wt = lst[g]
                for kl in range(GK):
                    kc = g * GK + kl
                    nc.tensor.matmul(
                        p0[:], lhsT=hT_sb[:, kc * B:(kc + 1) * B], rhs=wt[:, kl, :512],
                        start=(kc == 0), stop=(kc == 39),
                    )
                    nc.tensor.matmul(
                        p1[:], lhsT=hT_sb[:, kc * B:(kc + 1) * B], rhs=wt[:, kl, 512:],
                        start=(kc == 0), stop=(kc == 39),
                    )
            if scale == 1.0:
                nc.scalar.copy(nat[:, :512], p0[:])
                nc.scalar.copy(nat[:, 512:], p1[:])
            else:
                nc.scalar.mul(nat[:, :512], p0[:], scale)
                nc.scalar.mul(nat[:, 512:], p1[:], scale)

        def transpose_to(nat, dst_sb, i):
            tp = psA.tile([128, B], f16, tag="ps", name=f"tp{i}")
            nc.tensor.transpose(tp[:], nat[:, i * 128:(i + 1) * 128], ident[:])
            nc.scalar.copy(dst_sb[:, i * B:(i + 1) * B], tp[:])

        proj(wk_t, k_nat, rs_k)
        for i in range(HPC):
            transpose_to(k_nat, kT_sb, i)
        proj(wq_t, q_nat, 1.0)  # q pre-scaled by 1/sqrt(D) host-side
        for i in range(HPC):
            transpose_to(q_nat, qT_sb, i)
        proj(wv_t, v_nat, rs_v)

        # V new-token scatter: one contiguous DMA per sequence
        for b in range(B):
            p = pos[b]
            nc.sync.dma_start(
                out=newV[p % 128:p % 128 + 1, b * HPC:(b + 1) * HPC, :],
                in_=v_nat[b:b + 1, :],
            )

        # ---- per-slot attention (chase Kt/Vt arrivals) ----
        attn = {}
        for i in range(HPC):
            Kt = Kts[i]
            # K new-token scatter (same partitions: DVE copy)
            for b in range(B):
                lp = (kmoff[i][b] + pos[b] // 128 - c0[i][b]) * 128 + pos[b] % 128
                nc.vector.tensor_copy(
                    Kt[:, lp:lp + 1], kT_sb[:, i * B + b:i * B + b + 1]
                )
            for b in range(B):
                n = m[i][b]
                col = i * B + b
                sc_ps = psA.tile([128, 16], f32, tag="ps", name=f"sc_{i}_{b}")
                for c in range(n):
                    nc.tensor.matmul(
                        sc_ps[:, c:c + 1],
                        lhsT=Kt[:, (kmoff[i][b] + c) * 128:(kmoff[i][b] + c + 1) * 128],
                        rhs=qT_sb[:, col:col + 1],
                        start=True, stop=True,
                    )
                s_sb = consts.tile([128, 16], f32, name=f"s_{i}_{b}")
                nc.vector.tensor_add(
                    s_sb[:, :n], sc_ps[:, :n],
                    bias_sb[:, (b * HPC + i) * 16:(b * HPC + i) * 16 + n],
                )
                a_sb = consts.tile([128, 16], f16, name=f"at_{i}_{b}")
                nc.scalar.activation(
                    a_sb[:, :n], s_sb[:, :n],
                    func=mybir.ActivationFunctionType.Exp,
                    accum_out=colsum_sb[:, col:col + 1],
                )
                attn[(i, b)] = a_sb

        opsO = [psO.tile([B, 512], f32, tag="po", name=f"po{jg}") for jg in range(OP1)]
        for i in range(HPC):
            Vt = Vts[i]
            for b in range(B):
                n = m[i][b]
                col = i * B + b
                ao_ps = psA.tile([128, 1], f32, tag="ps", name=f"ao_{i}_{b}")
                for c in range(n - 1):
                    nc.tensor.matmul(
                        ao_ps[:],
                        lhsT=Vt[:, vmoff[i][b] + c, :],
                        rhs=attn[(i, b)][:, c:c + 1],
                        start=(c == 0), stop=False,
                    )
                nc.tensor.matmul(
                    ao_ps[:],
                    lhsT=newV[:, b * HPC + i, :],
                    rhs=attn[(i, b)][:, n - 1:n],
                    start=(n == 1), stop=True,
                )
                nc.scalar.copy(aoT_sb[:, col:col + 1], ao_ps[:])
            # per-slot normalization
            sums_ps = psA.tile([1, B], f32, tag="ps", name=f"sum{i}")
            nc.tensor.matmul(
                sums_ps[:], lhsT=ones_col[:],
                rhs=colsum_sb[:, i * B:(i + 1) * B], start=True, stop=True,
            )
            recip_sb = consts.tile([1, B], f32, name=f"rc{i}")
            nc.vector.reciprocal(recip_sb[:], sums_ps[:])
            rb_ps = psA.tile([128, B], f32, tag="ps", name=f"rb{i}")
            nc.tensor.matmul(
                rb_ps[:], lhsT=ones_row[:], rhs=recip_sb[:], start=True, stop=True,
            )
            rb_sb = consts.tile([128, B], f32, name=f"rbs{i}")
            nc.vector.tensor_copy(rb_sb[:], rb_ps[:])
            nc.vector.tensor_mul(
                attn_nT[:, i * B:(i + 1) * B], aoT_sb[:, i * B:(i + 1) * B], rb_sb[:]
            )
            # o_proj phase 1: slot-streamed accumulation
            for jg in range(OP1):
                nc.tensor.matmul(
                    opsO[jg][:],
                    lhsT=attn_nT[:, i * B:(i + 1) * B],
                    rhs=ow_ts[i][:, jg * 512:(jg + 1) * 512],
                    start=(i == 0), stop=(i == HPC - 1),
                )

        # ---- o_proj phase 1 evac + phase 2 ----
        for jg in range(OP1):
            if jg % 2 == 0:
                nc.scalar.copy(out_sb[:, jg * 512:(jg + 1) * 512], opsO[jg][:])
            else:
                nc.vector.tensor_copy(out_sb[:, jg * 512:(jg + 1) * 512], opsO[jg][:])
        for jg in range(OP1, E // 512):
            op2 = psO.tile([B, 512], f32, tag="po", name=f"po{jg}")
            for i in range(HPC):
                nc.tensor.matmul(
                    op2[:],
                    lhsT=attn_nT[:, i * B:(i + 1) * B],
                    rhs=ow_ts[i][:, jg * 512:(jg + 1) * 512],
                    start=(i == 0), stop=(i == HPC - 1),
                )
            if jg % 2 == 0:
                nc.scalar.copy(out_sb[:, jg * 512:(jg + 1) * 512], op2[:])
            else:
                nc.vector.tensor_copy(out_sb[:, jg * 512:(jg + 1) * 512], op2[:])

        nc.sync.dma_start(out=outT[:, :E // 2], in_=out_sb[:, :E // 2])
        nc.sync.dma_start(out=outT[:, E // 2:], in_=out_sb[:, E // 2:])

    nc.compile()
    return nc


def _pow2_scale(x, cap):
    mx = float(np.abs(x).max())
    return 2.0 ** math.floor(math.log2(cap / mx))


def _prepare_core_inputs(core, hidden16, wq16, wk8, wv8, o16, k16, v16, bt, sl,
                         pos, nch, head_ranks, m):
    """Per-core staged arrays; slot index = emission order."""
    heads = [head_ranks[i][core] for i in range(HPC)]

    def wlayout(Wh):  # Wh: [E, EPC] -> [128, 40, EPC]
        return np.ascontiguousarray(Wh.reshape(40, 128, EPC).transpose(1, 0, 2))

    wq = wlayout(wq16.reshape(E, H, D)[:, heads, :].reshape(E, EPC))
    wk = wlayout(wk8.reshape(E, H, D)[:, heads, :].reshape(E, EPC))
    wv = wlayout(wv8.reshape(E, H, D)[:, heads, :].reshape(E, EPC))

    mv = [[m[i][b] - 1 for b in range(B)] for i in range(HPC)]
    kcnt = [sum(m[i]) for i in range(HPC)]
    vcnt = [sum(mv[i]) for i in range(HPC)]
    ksoff = [0]
    vsoff = [0]
    for i in range(HPC):
        ksoff.append(ksoff[-1] + kcnt[i])
        vsoff.append(vsoff[-1] + vcnt[i])
    KCH, VCH = ksoff[-1], vsoff[-1]
    kmoff = [[sum(m[i][:b]) for b in range(B)] for i in range(HPC)]
    vmoff = [[sum(mv[i][:b]) for b in range(B)] for i in range(HPC)]
    c0 = [[nch[b] - m[i][b] for b in range(B)] for i in range(HPC)]

    kg = k16[:, heads]  # [NB, HPC, BS, D]
    vg = v16[:, heads]
    kt = np.zeros((D, KCH * 128), np.float16)
    vt = np.zeros((128, max(VCH, 1), D), np.float16)
    nv = np.zeros((128, B * HPC, D), np.float16)
    for b in range(B):
        sd = nch[b] * 128
        blocks = bt[b][: sd // BS]
        kk = kg[blocks].transpose(1, 0, 2, 3).reshape(HPC, sd, D)
        vv = vg[blocks].transpose(1, 0, 2, 3).reshape(HPC, sd, D)
        for i in range(HPC):
            base = ksoff[i] + kmoff[i][b]
            n = m[i][b]
            ksl = kk[i, c0[i][b] * 128: sd]              # [n*128, D]
            kt[:, base * 128:(base + n) * 128] = ksl.T
            vsl = vv[i, c0[i][b] * 128: sd].reshape(n, 128, D)
            vb = vsoff[i] + vmoff[i][b]
            vt[:, vb:vb + n - 1, :] = vsl[:-1].transpose(1, 0, 2)
            nv[:, b * HPC + i, :] = vsl[-1]

    slopes = _alibi_slopes(H)[heads]
    t_in = np.arange(128)[:, None]
    biasa = np.full((128, B, HPC, 16), NEG, np.float32)
    for b in range(B):
        for i in range(HPC):
            n = m[i][b]
            tg = ((c0[i][b] + np.arange(n))[None, :] * 128 + t_in).astype(np.float32)
            val = slopes[i] * (tg - np.float32(pos[b]))
            val[tg >= sl[b]] = NEG
            biasa[:, b, i, :n] = val

    hTf = np.ascontiguousarray(
        hidden16.T.reshape(40, 128, B).transpose(1, 0, 2).reshape(128, 40 * B)
    )
    owr = np.ascontiguousarray(o16.reshape(H, D, E)[heads])

    return dict(
        hT=hTf, wq=wq, wk=wk, wv=wv, ow=owr, kt=kt, vt=vt, nv=nv,
        bias=np.ascontiguousarray(biasa.reshape(128, B * HPC * 16)),
    )


def kernel(**inputs):
    global LAST_RESULTS
    hidden = np.asarray(inputs["hidden_states"], np.float32)
    qkv_w = np.asarray(inputs["qkv_weight"], np.float32)
    o_w = np.asarray(inputs["o_proj_weight"], np.float32)
    k_cache = np.asarray(inputs["k_cache"], np.float32)
    v_cache = np.asarray(inputs["v_cache"], np.float32)
    bt = np.asarray(inputs["block_tables"]).astype(np.int64)
    sl = np.asarray(inputs["sequence_lengths"]).astype(np.int64)

    pos = tuple(int(x) - 1 for x in sl)
    nch = tuple(int(math.ceil(int(x) / 128)) for x in sl)
    head_ranks, m = _head_partition(pos, nch)

    hidden16 = hidden.astype(np.float16)
    wq16 = (qkv_w[0] * np.float32(D ** -0.5)).astype(np.float16)
    s_k = _pow2_scale(qkv_w[1], 14.0)
    s_v = _pow2_scale(qkv_w[2], 14.0)
    wk8 = (qkv_w[1] * np.float32(s_k)).astype(E3)
    wv8 = (qkv_w[2] * np.float32(s_v)).astype(E3)
    o16 = o_w.astype(np.float16)
    k16 = k_cache.astype(np.float16)
    v16 = v_cache.astype(np.float16)

    in_maps = [
        _prepare_core_inputs(c, hidden16, wq16, wk8, wv8, o16, k16, v16, bt, sl,
                             pos, nch, head_ranks, m)
        for c in range(NCORES)
    ]

    key = (pos, nch, m, s_k, s_v)
    if key not in _PROGRAM_CACHE:
        _PROGRAM_CACHE[key] = _build_program(pos, nch, m, 1.0 / s_k, 1.0 / s_v)
    nc = _PROGRAM_CACHE[key]

    from concourse.bass_utils import run_bass_kernel_spmd

    res = run_bass_kernel_spmd(
        nc,
        in_maps,
        core_ids=list(range(NCORES)),
        trace=bool(os.environ.get("BASS_TRACE")),
    )
    LAST_RESULTS = res

    out = np.zeros((B, E), np.float64)
    for c in range(NCORES):
        out += np.asarray(res.results[c]["outT"]).astype(np.float64)
    return out.astype(np.float32)


# revision 14
# speedup vs baseline: 1.4177x; 1.1281x over previous
"""Paged KV-cache decode attention with ALiBi (Baichuan-style), fused
QKV + attention + output projection, tensor-parallel over heads across
8 Trainium2 NeuronCores.

v9: dense PE schedule + contiguous V new-token scatter.

Per core (5 head-slots, slot index = emission order = descending alibi
window so the small slots land in the tail):
  - all projections in natural orientation (lhsT = hT chunk [128(E),4],
    rhs = W chunk [128(E), 512/128] streaming) -> PE at the stream
    floor (~10.9us each) instead of LDWEIGHTS-bound.
  - emission k -> q -> v keeps the PE on wide streams back-to-back
    (HAM stays at 2.4GHz); scores/AV (N=1 matmuls) chase the K/V tile
    DMAs afterwards, interleaved with N=512 o_proj matmuls.
  - qT/kT via PE transposes ([4,128] -> [128,4]) with prompt PSUM
    evacuation (short tile lifetimes - avoids cross-engine stalls).
  - wk/wv quantized host-side to fp8 e3m4 (power-of-2 scale, descale
    folded into the PSUM->SBUF evac). rel_err ~9e-3 vs 2e-2 gate.
  - V new-token scatter: the last (newest) chunk of every (slot, seq)
    lives in a separate newV tile [128, B*HPC, 128] packed so that the
    new-token row for seq b is CONTIGUOUS -> 4 DMAs total instead of
    20 serial ones (v8 lost ~17us here).
  - o_proj split: 5 groups accumulate slot-by-slot in 5 PSUM banks as
    each slot normalizes (per-slot softmax normalization); 5 groups
    replay all slots at the end. Tail ~6us instead of ~11us.
  - DMA order: hT, bias, wk, wq, wv, Kt(slots), newV, [Vt(i), ow(i)].
"""

import math
import os
import sys
from contextlib import ExitStack

import numpy as np
import ml_dtypes

sys.path.insert(0, "/opt/trn_rl_repo")

B = 4
E = 5120
H = 40
D = 128
BS = 16
NB = 512
MB = 128
S = MB * BS  # 2048
NCORES = 8
HPC = H // NCORES   # 5 head-slots per core
EPC = HPC * D       # 640

NEG = -1.0e30
GK = 10             # E-chunks (of 128) per qkv weight DMA group
NG = 40 // GK
TCUT = 12.0         # alibi bias cutoff (dropped weight <= ~e^-12 rel)

E3 = ml_dtypes.float8_e3m4


def _alibi_slopes(num_heads):
    cp2 = 2 ** int(math.floor(math.log2(num_heads)))
    base = 2.0 ** (-(2.0 ** (-(math.log2(cp2) - 3))))
    slopes = base ** np.arange(1, cp2 + 1, dtype=np.float64)
    if cp2 != num_heads:
        extra_base = 2.0 ** (-(2.0 ** (-(math.log2(2 * cp2) - 3))))
        n_rem = min(cp2, num_heads - cp2)
        extra = extra_base ** np.arange(1, 1 + 2 * n_rem, 2, dtype=np.float64)
        slopes = np.concatenate([slopes, extra])
    return slopes.astype(np.float32)


def _head_partition(pos, nch):
    """Rank heads by alibi window; slot i (in emission order, biggest
    window first) gets ranks [gi*8, (gi+1)*8) where gi runs over groups
    sorted by descending window. Returns (head_ranks, m) with
    head_ranks[i] = the 8 global head ids of slot i (per-core pick is
    head_ranks[i][core]) and m[i][b] = kept trailing chunks."""
    win = np.ceil(TCUT / _alibi_slopes(H).astype(np.float64)).astype(np.int64)
    order = np.argsort(win, kind="stable")
    groups = []
    for g in range(HPC):
        ids = order[g * NCORES:(g + 1) * NCORES]
        wmax = int(win[ids].max())
        groups.append((wmax, [int(x) for x in ids]))
    groups.sort(key=lambda t: -t[0])
    head_ranks = tuple(tuple(ids) for _, ids in groups)
    m = tuple(
        tuple(nch[b] - max(0, (pos[b] - wmax) // 128) for b in range(B))
        for wmax, _ in groups
    )
    return head_ranks, m


_PROGRAM_CACHE = {}
LAST_RESULTS = None  # BassKernelResults of the most recent run (for test.py)


def _build_program(pos, nch, m, rs_k, rs_v):
    """Build the SPMD Bass program. pos/nch/m and the fp8 descale
    factors are baked statically (same for all cores)."""
    import concourse.bacc as bacc
    import concourse.bass as bass
    import concourse.tile as tile
    from concourse import mybir

    f32 = mybir.dt.float32
    f16 = mybir.dt.float16
    f8 = mybir.dt.float8e3
    nc = bacc.Bacc()

    # Vt holds chunks 0..m-2 per (i,b); the last chunk lives in newV.
    mv = [[m[i][b] - 1 for b in range(B)] for i in range(HPC)]
    vcnt = [sum(mv[i]) for i in range(HPC)]
    kcnt = [sum(m[i]) for i in range(HPC)]
    ksoff = [0]
    vsoff = [0]
    for i in range(HPC):
        ksoff.append(ksoff[-1] + kcnt[i])
        vsoff.append(vsoff[-1] + vcnt[i])
    KCH, VCH = ksoff[-1], vsoff[-1]
    kmoff = [[sum(m[i][:b]) for b in range(B)] for i in range(HPC)]
    vmoff = [[sum(mv[i][:b]) for b in range(B)] for i in range(HPC)]
    c0 = [[nch[b] - m[i][b] for b in range(B)] for i in range(HPC)]

    hT = nc.declare_dram_parameter("hT", [128, 40 * B + B], f16, isOutput=False)
    wq_d = nc.declare_dram_parameter("wq", [128, 40, EPC], f16, isOutput=False)
    wk_d = nc.declare_dram_parameter("wk", [128, 40, EPC], f8, isOutput=False)
    wv_d = nc.declare_dram_parameter("wv", [128, 40, EPC], f8, isOutput=False)
    ow_d = nc.declare_dram_parameter("ow", [HPC, 128, E], f16, isOutput=False)
    kt_d = nc.declare_dram_parameter("kt", [128, KCH * 128], f16, isOutput=False)
    vt_d = nc.declare_dram_parameter("vt", [128, max(VCH, 1), D], f16, isOutput=False)
    nv_d = nc.declare_dram_parameter("nv", [128, B * HPC, D], f16, isOutput=False)
    bias_d = nc.declare_dram_parameter("bias", [128, B * HPC * 16], f32, isOutput=False)
    outT = nc.declare_dram_parameter("outT", [B, E], f32, isOutput=True)

    with tile.TileContext(nc) as tc, ExitStack() as ctx:
        consts = ctx.enter_context(tc.tile_pool(name="consts", bufs=1))
        wpool = ctx.enter_context(tc.tile_pool(name="wpool", bufs=5))
        psA = ctx.enter_context(tc.tile_pool(name="psA", bufs=3, space="PSUM"))
        psO = ctx.enter_context(tc.tile_pool(name="psO", bufs=5, space="PSUM"))

        hT_sb = consts.tile([128, 40 * B + B], f16)
        bias_sb = consts.tile([128, B * HPC * 16], f32)
        # 4x4 identity for PE transposes rides in the last hT columns
        ident = hT_sb[0:B, 40 * B:40 * B + B]
        ones_col = consts.tile([128, 1], f32)
        nc.vector.memset(ones_col[:], 1.0)
        ones_row = consts.tile([1, 128], f32)
        nc.vector.memset(ones_row[:], 1.0)

        qT_sb = consts.tile([128, HPC * B], f16)    # [d, i*B+b]
        kT_sb = consts.tile([128, HPC * B], f16)
        colsum_sb = consts.tile([128, HPC * B], f32)
        aoT_sb = consts.tile([128, HPC * B], f32)
        attn_nT = consts.tile([128, HPC * B], f16)
        q_nat = consts.tile([B, EPC], f16)
        k_nat = consts.tile([B, EPC], f16)
        v_nat = consts.tile([B, EPC], f16)
        # o_proj output: jg 0-4 on partitions 0-3, jg 5-9 on 64-67
        out_sb = consts.tile([128, E // 2], f32)

        Kts = [consts.tile([128, kcnt[i] * 128], f16, name=f"K{i}") for i in range(HPC)]
        Vts = [consts.tile([128, max(vcnt[i], 1), D], f16, name=f"V{i}")
               for i in range(HPC)]
        newV = consts.tile([128, B * HPC, D], f16)
        ow_ts = [consts.tile([128, E], f16, name=f"ow{i}") for i in range(HPC)]

        # ---- bulk DMA stream on the gpsimd (SWDGE) queue, in order ----
        nc.gpsimd.dma_start(out=hT_sb[:], in_=hT[:])
        nc.gpsimd.dma_start(out=bias_sb[:], in_=bias_d[:])
        wq_t, wk_t, wv_t = [], [], []

        def wgroup(dram, lst, nm, dt):
            t = wpool.tile([128, GK, EPC], dt, tag="w", name=f"{nm}{len(lst)}")
            nc.gpsimd.dma_start(out=t[:], in_=dram[:, len(lst) * GK:(len(lst) + 1) * GK, :])
            lst.append(t)

        for g in range(NG):
            wgroup(wk_d, wk_t, "wk", f8)
        for g in range(NG):
            wgroup(wq_d, wq_t, "wq", f16)
        for g in range(NG):
            wgroup(wv_d, wv_t, "wv", f8)
        for i in range(HPC):
            nc.gpsimd.dma_start(
                out=Kts[i][:], in_=kt_d[:, ksoff[i] * 128:ksoff[i + 1] * 128]
            )
        nc.gpsimd.dma_start(out=newV[:], in_=nv_d[:])
        for i in range(HPC):
            if vcnt[i]:
                nc.gpsimd.dma_start(
                    out=Vts[i][:], in_=vt_d[:, vsoff[i]:vsoff[i + 1], :]
                )
            nc.gpsimd.dma_start(out=ow_ts[i][:], in_=ow_d[i, :, :])

        # ---- projections (natural orientation) ----
        def proj(lst, nat, scale):
            p0 = psA.tile([B, 512], f32, tag="ps")
            p1 = psA.tile([B, EPC - 512], f32, tag="ps")
            for g in range(NG):
                wt = lst[g]
                for kl in range(GK):
                    kc = g * GK + kl
                    nc.tensor.matmul(
                        p0[:], lhsT=hT_sb[:, kc * B:(kc + 1) * B], rhs=wt[:, kl, :512],
                        start=(kc == 0), stop=(kc == 39),
                    )
                    nc.tensor.matmul(
                        p1[:], lhsT=hT_sb[:, kc * B:(kc + 1) * B], rhs=wt[:, kl, 512:],
                        start=(kc == 0), stop=(kc == 39),
                    )
            if scale == 1.0:
                nc.scalar.copy(nat[:, :512], p0[:])
                nc.scalar.copy(nat[:, 512:], p1[:])
            else:
                nc.scalar.mul(nat[:, :512], p0[:], scale)
                nc.scalar.mul(nat[:, 512:], p1[:], scale)

        def transpose_to(nat, dst_sb, i):
            tp = psA.tile([128, B], f16, tag="ps", name=f"tp{i}")
            nc.tensor.transpose(tp[:], nat[:, i * 128:(i + 1) * 128], ident[:])
            nc.scalar.copy(dst_sb[:, i * B:(i + 1) * B], tp[:])

        proj(wk_t, k_nat, rs_k)
        for i in range(HPC):
            transpose_to(k_nat, kT_sb, i)
        proj(wq_t, q_nat, 1.0)  # q pre-scaled by 1/sqrt(D) host-side
        for i in range(HPC):
            transpose_to(q_nat, qT_sb, i)
        proj(wv_t, v_nat, rs_v)

        # V new-token scatter: one contiguous DMA per sequence
        for b in range(B):
            p = pos[b]
            nc.sync.dma_start(
                out=newV[p % 128:p % 128 + 1, b * HPC:(b + 1) * HPC, :],
                in_=v_nat[b:b + 1, :],
            )

        # ---- per-slot attention (chase Kt/Vt arrivals) ----
        attn = {}
        for i in range(HPC):
            Kt = Kts[i]
            # K new-token scatter (same partitions: DVE copy)
            for b in range(B):
                lp = (kmoff[i][b] + pos[b] // 128 - c0[i][b]) * 128 + pos[b] % 128
                nc.vector.tensor_copy(
                    Kt[:, lp:lp + 1], kT_sb[:, i * B + b:i * B + b + 1]
                )
            for b in range(B):
                n = m[i][b]
                col = i * B + b
                sc_ps = psA.tile([128, 16], f32, tag="ps", name=f"sc_{i}_{b}")
                for c in range(n):
                    nc.tensor.matmul(
                        sc_ps[:, c:c + 1],
                        lhsT=Kt[:, (kmoff[i][b] + c) * 128:(kmoff[i][b] + c + 1) * 128],
                        rhs=qT_sb[:, col:col + 1],
                        start=True, stop=True,
                    )
                s_sb = consts.tile([128, 16], f32, name=f"s_{i}_{b}")
                nc.vector.tensor_add(
                    s_sb[:, :n], sc_ps[:, :n],
                    bias_sb[:, (b * HPC + i) * 16:(b * HPC + i) * 16 + n],
                )
                a_sb = consts.tile([128, 16], f16, name=f"at_{i}_{b}")
                nc.scalar.activation(
                    a_sb[:, :n], s_sb[:, :n],
                    func=mybir.ActivationFunctionType.Exp,
                    accum_out=colsum_sb[:, col:col + 1],
                )
                attn[(i, b)] = a_sb

        # o_proj accumulators: 10 jg groups packed 2-per-bank (col tiling:
        # jg<5 at partitions 0-3, jg>=5 at partitions 64-67 of bank jg%5)
        opsO_t = [psO.tile([128, 512], f32, tag="po", name=f"po{t}") for t in range(5)]

        def o_acc(jg):
            t = opsO_t[jg % 5]
            return (t[0:B, :], (0, 0)) if jg < 5 else (t[64:64 + B, :], (0, 64))
        for i in range(HPC):
            Vt = Vts[i]
            for b in range(B):
                n = m[i][b]
                col = i * B + b
                ao_ps = psA.tile([128, 1], f32, tag="ps", name=f"ao_{i}_{b}")
                for c in range(n - 1):
                    nc.tensor.matmul(
                        ao_ps[:],
                        lhsT=Vt[:, vmoff[i][b] + c, :],
                        rhs=attn[(i, b)][:, c:c + 1],
                        start=(c == 0), stop=False,
                    )
                nc.tensor.matmul(
                    ao_ps[:],
                    lhsT=newV[:, b * HPC + i, :],
                    rhs=attn[(i, b)][:, n - 1:n],
                    start=(n == 1), stop=True,
                )
                nc.scalar.copy(aoT_sb[:, col:col + 1], ao_ps[:])
            # per-slot normalization
            sums_ps = psA.tile([1, B], f32, tag="ps", name=f"sum{i}")
            nc.tensor.matmul(
                sums_ps[:], lhsT=ones_col[:],
                rhs=colsum_sb[:, i * B:(i + 1) * B], start=True, stop=True,
            )
            recip_sb = consts.tile([1, B], f32, name=f"rc{i}")
            nc.vector.reciprocal(recip_sb[:], sums_ps[:])
            rb_ps = psA.tile([128, B], f32, tag="ps", name=f"rb{i}")
            nc.tensor.matmul(
                rb_ps[:], lhsT=ones_row[:], rhs=recip_sb[:], start=True, stop=True,
            )
            rb_sb = consts.tile([128, B], f32, name=f"rbs{i}")
            nc.vector.tensor_copy(rb_sb[:], rb_ps[:])
            nc.vector.tensor_mul(
                attn_nT[:, i * B:(i + 1) * B], aoT_sb[:, i * B:(i + 1) * B], rb_sb[:]
            )
            # o_proj: all 10 jg groups slot-streamed (2 accumulators/bank)
            for jg in range(E // 512):
                acc, tpos = o_acc(jg)
                nc.tensor.matmul(
                    acc,
                    lhsT=attn_nT[:, i * B:(i + 1) * B],
                    rhs=ow_ts[i][:, jg * 512:(jg + 1) * 512],
                    start=(i == 0), stop=(i == HPC - 1),
                    tile_position=tpos,
                )

        # ---- o_proj evac + stores ----
        for jg in range(E // 512):
            acc, _ = o_acc(jg)
            dst = out_sb[0:B, (jg % 5) * 512:(jg % 5 + 1) * 512] if jg < 5 else \
                out_sb[64:64 + B, (jg % 5) * 512:(jg % 5 + 1) * 512]
            if jg % 2 == 0:
                nc.scalar.copy(dst, acc)
            else:
                nc.vector.tensor_copy(dst, acc)

        nc.sync.dma_start(out=outT[:, :E // 2], in_=out_sb[0:B, :])
        nc.sync.dma_start(out=outT[:, E // 2:], in_=out_sb[64:64 + B, :])

    nc.compile()
    return nc


def _pow2_scale(x, cap):
    mx = float(np.abs(x).max())
    return 2.0 ** math.floor(math.log2(cap / mx))


def _prepare_core_inputs(core, hidden16, wq16, wk8, wv8, o16, k16, v16, bt, sl,
                         pos, nch, head_ranks, m):
    """Per-core staged arrays; slot index = emission order."""
    heads = [head_ranks[i][core] for i in range(HPC)]

    def wlayout(Wh):  # Wh: [E, EPC] -> [128, 40, EPC]
        return np.ascontiguousarray(Wh.reshape(40, 128, EPC).transpose(1, 0, 2))

    wq = wlayout(wq16.reshape(E, H, D)[:, heads, :].reshape(E, EPC))
    wk = wlayout(wk8.reshape(E, H, D)[:, heads, :].reshape(E, EPC))
    wv = wlayout(wv8.reshape(E, H, D)[:, heads, :].reshape(E, EPC))

    mv = [[m[i][b] - 1 for b in range(B)] for i in range(HPC)]
    kcnt = [sum(m[i]) for i in range(HPC)]
    vcnt = [sum(mv[i]) for i in range(HPC)]
    ksoff = [0]
    vsoff = [0]
    for i in range(HPC):
        ksoff.append(ksoff[-1] + kcnt[i])
        vsoff.append(vsoff[-1] + vcnt[i])
    KCH, VCH = ksoff[-1], vsoff[-1]
    kmoff = [[sum(m[i][:b]) for b in range(B)] for i in range(HPC)]
    vmoff = [[sum(mv[i][:b]) for b in range(B)] for i in range(HPC)]
    c0 = [[nch[b] - m[i][b] for b in range(B)] for i in range(HPC)]

    kg = k16[:, heads]  # [NB, HPC, BS, D]
    vg = v16[:, heads]
    kt = np.zeros((D, KCH * 128), np.float16)
    vt = np.zeros((128, max(VCH, 1), D), np.float16)
    nv = np.zeros((128, B * HPC, D), np.float16)
    for b in range(B):
        sd = nch[b] * 128
        blocks = bt[b][: sd // BS]
        kk = kg[blocks].transpose(1, 0, 2, 3).reshape(HPC, sd, D)
        vv = vg[blocks].transpose(1, 0, 2, 3).reshape(HPC, sd, D)
        for i in range(HPC):
            base = ksoff[i] + kmoff[i][b]
            n = m[i][b]
            ksl = kk[i, c0[i][b] * 128: sd]              # [n*128, D]
            kt[:, base * 128:(base + n) * 128] = ksl.T
            vsl = vv[i, c0[i][b] * 128: sd].reshape(n, 128, D)
            vb = vsoff[i] + vmoff[i][b]
            vt[:, vb:vb + n - 1, :] = vsl[:-1].transpose(1, 0, 2)
            nv[:, b * HPC + i, :] = vsl[-1]

    slopes = _alibi_slopes(H)[heads]
    t_in = np.arange(128)[:, None]
    biasa = np.full((128, B, HPC, 16), NEG, np.float32)
    for b in range(B):
        for i in range(HPC):
            n = m[i][b]
            tg = ((c0[i][b] + np.arange(n))[None, :] * 128 + t_in).astype(np.float32)
            val = slopes[i] * (tg - np.float32(pos[b]))
            val[tg >= sl[b]] = NEG
            biasa[:, b, i, :n] = val

    hTf = np.zeros((128, 40 * B + B), np.float16)
    hTf[:, :40 * B] = hidden16.T.reshape(40, 128, B).transpose(1, 0, 2).reshape(
        128, 40 * B)
    hTf[:B, 40 * B:] = np.eye(B, dtype=np.float16)
    owr = np.ascontiguousarray(o16.reshape(H, D, E)[heads])

    return dict(
        hT=hTf, wq=wq, wk=wk, wv=wv, ow=owr, kt=kt, vt=vt, nv=nv,
        bias=np.ascontiguousarray(biasa.reshape(128, B * HPC * 16)),
    )


def kernel(**inputs):
    global LAST_RESULTS
    hidden = np.asarray(inputs["hidden_states"], np.float32)
    qkv_w = np.asarray(inputs["qkv_weight"], np.float32)
    o_w = np.asarray(inputs["o_proj_weight"], np.float32)
    k_cache = np.asarray(inputs["k_cache"], np.float32)
    v_cache = np.asarray(inputs["v_cache"], np.float32)
    bt = np.asarray(inputs["block_tables"]).astype(np.int64)
    sl = np.asarray(inputs["sequence_lengths"]).astype(np.int64)

    pos = tuple(int(x) - 1 for x in sl)
    nch = tuple(int(math.ceil(int(x) / 128)) for x in sl)
    head_ranks, m = _head_partition(pos, nch)

    hidden16 = hidden.astype(np.float16)
    wq16 = (qkv_w[0] * np.float32(D ** -0.5)).astype(np.float16)
    s_k = _pow2_scale(qkv_w[1], 14.0)
    s_v = _pow2_scale(qkv_w[2], 14.0)
    wk8 = (qkv_w[1] * np.float32(s_k)).astype(E3)
    wv8 = (qkv_w[2] * np.float32(s_v)).astype(E3)
    o16 = o_w.astype(np.float16)
    k16 = k_cache.astype(np.float16)
    v16 = v_cache.astype(np.float16)

    in_maps = [
        _prepare_core_inputs(c, hidden16, wq16, wk8, wv8, o16, k16, v16, bt, sl,
                             pos, nch, head_ranks, m)
        for c in range(NCORES)
    ]

    key = (pos, nch, m, s_k, s_v)
    if key not in _PROGRAM_CACHE:
        _PROGRAM_CACHE[key] = _build_program(pos, nch, m, 1.0 / s_k, 1.0 / s_v)
    nc = _PROGRAM_CACHE[key]

    from concourse.bass_utils import run_bass_kernel_spmd

    res = run_bass_kernel_spmd(
        nc,
        in_maps,
        core_ids=list(range(NCORES)),
        trace=bool(os.environ.get("BASS_TRACE")),
    )
    LAST_RESULTS = res

    out = np.zeros((B, E), np.float64)
    for c in range(NCORES):
        out += np.asarray(res.results[c]["outT"]).astype(np.float64)
    return out.astype(np.float32)
